# revision 1
# baseline (speedup 1.0000x reference)
"""ChannelAttn Trainium2 kernel v2: N-sharded SPMD with Gram-matrix scores.

Sharding: core c handles batch b=c//4, rows [2048*(c%4), 2048*(c%4+1)),
and owns head h=c%4 for the score/softmax work. Replica groups per batch:
[[0,1,2,3],[4,5,6,7]].

Score path (needs fp32 accuracy; softmax cols have top-2 gaps >= 0.25):
  G = x~^T x~ per batch where x~ = xh + xl (bf16 hi/lo split of x, done
  host-side). M = 1/2 xh^T xh + xh^T xl is computed as
  xh^T xh + xh^T xl2 (xl2 = 2*xl, exact) then scaled 0.5 on the
  PSUM->SBUF copy; AllReduce(M) over the batch group in two column
  halves (pipelined behind compute); G = M + M^T locally.
  S^T_h = wk_h G wq_h^T in true fp32 matmuls (own head only), softmax
  over free dim, transpose to sm[d,e] bf16, AllGather within group.

Value path (bf16; each stage contributes ~2e-3 rms, tolerance 2e-2):
  xhT = transpose(xh); vT[(h,d), n] = wv_arr^T-contract xhT;
  x_caT_h[e, n] = sum_d sm_h[d, e] vT_h[d, n];
  out[n, :] = sum_{h,e} x_caT_h[e, n] wp[(h,e), :] + bias.

Shapes hardcoded: B=2, N=8192, C=768, H=4, HD=192.
"""

import sys

sys.path.insert(0, "/opt/trn_rl_repo")

import numpy as np

B, N, C, H = 2, 8192, 768, 4
HD = C // H  # 192
NCORE = 8
ROWS = N // 4  # 2048 rows per core (of one batch)
NCH = ROWS // 128  # 16 chunks
NW = ROWS // 512  # 4 windows
NT = ROWS // 128  # 16 out tiles

_cached = {}


def _build():
    import concourse.bacc as bacc
    import concourse.mybir as mybir
    import concourse.tile as tile

    f32 = mybir.dt.float32
    bf16 = mybir.dt.bfloat16

    nc = bacc.Bacc("TRN2", target_bir_lowering=False, debug=False)

    xh_d = nc.dram_tensor("xh", [ROWS, C], bf16, kind="ExternalInput")
    xl2_d = nc.dram_tensor("xl2", [ROWS, C], bf16, kind="ExternalInput")
    wvlo_d = nc.dram_tensor("wvlo", [C, 512], bf16, kind="ExternalInput")
    wvhi_d = nc.dram_tensor("wvhi", [C, 256], bf16, kind="ExternalInput")
    wqt_d = nc.dram_tensor("wqt", [C, C], f32, kind="ExternalInput")
    wkt_d = nc.dram_tensor("wkt", [C, C], f32, kind="ExternalInput")
    wpt_d = nc.dram_tensor("wpt", [C, C], bf16, kind="ExternalInput")
    bias_d = nc.dram_tensor("bias", [128, C], f32, kind="ExternalInput")
    tvec_d = nc.dram_tensor("tvec", [128, 4], f32, kind="ExternalInput")
    identb_d = nc.dram_tensor("identb", [128, 128], bf16, kind="ExternalInput")
    identf_d = nc.dram_tensor("identf", [128, 128], f32, kind="ExternalInput")
    out_d = nc.dram_tensor("out", [2, 1024, C], f32, kind="ExternalOutput")

    RG = [[0, 1, 2, 3], [4, 5, 6, 7]]

    with tile.TileContext(nc) as tc:
        with (
            tc.tile_pool(name="wpool", bufs=1) as wpool,
            tc.tile_pool(name="drpool", bufs=1, space="DRAM") as drpool,
        ):
            vt_cm = tc.tile_pool(name="vtpool", bufs=1)
            vtpool = vt_cm.__enter__()
            xht_cm = tc.tile_pool(name="xhtpool", bufs=1)
            xhtpool = xht_cm.__enter__()
            small_cm = tc.tile_pool(name="smallpool", bufs=1)
            smallpool = small_cm.__enter__()

            # ---- x hi/lo to SBUF first (short-lived pool, closed after T) --
            xio_cm = tc.tile_pool(name="xio", bufs=1)
            xio = xio_cm.__enter__()
            xh_s = xio.tile([128, NCH, C], bf16, name="xh_s")
            xl2_s = xio.tile([128, NCH, C], bf16, name="xl2_s")
            for g in range(4):
                sl = slice(512 * g, 512 * (g + 1))
                nc.sync.dma_start(
                    xh_s[:, 4 * g : 4 * (g + 1), :],
                    xh_d[sl, :].rearrange("(o p) f -> p o f", p=128),
                )
                nc.sync.dma_start(
                    xl2_s[:, 4 * g : 4 * (g + 1), :],
                    xl2_d[sl, :].rearrange("(o p) f -> p o f", p=128),
                )

            # ---- weights / constants ----
            wvlo_s = wpool.tile([128, 6, 512], bf16)
            nc.sync.dma_start(
                wvlo_s[:], wvlo_d.ap().rearrange("(o p) f -> p o f", p=128)
            )
            wvhi_s = wpool.tile([128, 6, 256], bf16)
            nc.sync.dma_start(
                wvhi_s[:], wvhi_d.ap().rearrange("(o p) f -> p o f", p=128)
            )
            wpt_s = wpool.tile([128, 6, C], bf16)
            nc.sync.dma_start(
                wpt_s[:], wpt_d.ap().rearrange("(o p) f -> p o f", p=128)
            )
            bias_s = wpool.tile([128, C], f32)
            nc.sync.dma_start(bias_s[:], bias_d[:])
            tvec_s = wpool.tile([128, 4], f32)
            nc.sync.dma_start(tvec_s[:], tvec_d[:])
            identb = wpool.tile([128, 128], bf16)
            nc.sync.dma_start(identb[:], identb_d[:])
            identf = wpool.tile([128, 128], f32)
            nc.sync.dma_start(identf[:], identf_d[:])


            ssend = drpool.tile([4, HD, HD], f32)
            srecv = drpool.tile([4, HD, HD], f32)
            a2asend = drpool.tile([8, 4, 24, ROWS], bf16)
            a2arecv = drpool.tile([8, 4, 24, ROWS], bf16)
            vband = drpool.tile([2, 96, N], bf16)

            # ============ Phase G: M = xh^T xh + xh^T xl2 (column halves) ====
            msb = smallpool.tile([128, 6, C], f32, name="msb")
            with (
                tc.tile_pool(name="ps_g", bufs=1, space="PSUM") as ps_g,
            ):
                for half in range(2):
                    csl = slice(384 * half, 384 * (half + 1))
                    mps = [
                        ps_g.tile([128, 384], f32, tag=f"m{j}", name=f"mps{j}")
                        for j in range(6)
                    ]
                    for ch in range(NCH):
                        for j in range(6):
                            lhs = xh_s[:, ch, 128 * j : 128 * (j + 1)]
                            nc.tensor.matmul(
                                mps[j][:], lhs, xh_s[:, ch, csl],
                                start=(ch == 0), stop=False,
                                skip_group_check=True,
                            )
                            nc.tensor.matmul(
                                mps[j][:], lhs, xl2_s[:, ch, csl],
                                start=False, stop=(ch == NCH - 1),
                                skip_group_check=True,
                            )
                    for j in range(6):
                        if j % 2 == 0:
                            nc.vector.tensor_scalar_mul(
                                msb[:, j, csl], mps[j][:], 0.5
                            )
                        else:
                            nc.scalar.mul(msb[:, j, csl], mps[j][:], 0.5)

            # ============ Phase T: transpose xh -> xhT ============
            xht_s = xhtpool.tile([128, 6, ROWS], bf16, name="xht_s")
            with tc.tile_pool(name="ps_t", bufs=2, space="PSUM") as ps_t:
                for ch in range(NCH):
                    tp = ps_t.tile([128, C], bf16, tag="tp")
                    for j in range(6):
                        nc.tensor.matmul(
                            tp[:, 128 * j : 128 * (j + 1)],
                            xh_s[:, ch, 128 * j : 128 * (j + 1)],
                            identb[:], is_transpose=True,
                            start=True, stop=True, skip_group_check=True,
                        )
                    if ch % 2 == 0:
                        nc.vector.tensor_copy(
                            xht_s[:, :, 128 * ch : 128 * (ch + 1)],
                            tp[:].rearrange("p (o f) -> p o f", f=128),
                        )
                    else:
                        nc.scalar.copy(
                            xht_s[:, :, 128 * ch : 128 * (ch + 1)],
                            tp[:].rearrange("p (o f) -> p o f", f=128),
                        )

            xio_cm.__exit__(None, None, None)

            # ============ Phase S: G = M + M^T local, 4-head partial scores ==
            g_s = smallpool.tile([128, 6, C], f32)
            with tc.tile_pool(name="ps_sym", bufs=2, space="PSUM") as ps_sym:
                for i in range(6):
                    trow = ps_sym.tile([128, C], f32, tag="trow")
                    for j in range(6):
                        nc.tensor.matmul(
                            trow[:, 128 * j : 128 * (j + 1)],
                            msb[:, j, 128 * i : 128 * (i + 1)],
                            identf[:], is_transpose=True,
                            start=True, stop=True, skip_group_check=True,
                        )
                    nc.vector.tensor_add(g_s[:, i, :], msb[:, i, :], trow[:])

            sp_cm = tc.tile_pool(name="spool", bufs=1)
            spool = sp_cm.__enter__()
            wqt_s = spool.tile([128, 6, C], f32, name="wqt_s")
            nc.sync.dma_start(
                wqt_s[:], wqt_d.ap().rearrange("(o p) f -> p o f", p=128)
            )
            wkt_s = spool.tile([128, 6, C], f32, name="wkt_s")
            nc.sync.dma_start(
                wkt_s[:], wkt_d.ap().rearrange("(o p) f -> p o f", p=128)
            )
            a_s = spool.tile([128, 6, HD], f32)
            sp_lo = spool.tile([128, 4, HD], f32)
            sp_hi = spool.tile([64, 4, HD], f32)
            for h in range(4):
                hsl = slice(HD * h, HD * (h + 1))
                with tc.tile_pool(name=f"ps_a{h}", bufs=1, space="PSUM") as ps_a:
                    for i in range(6):
                        ap_t = ps_a.tile([128, HD], f32, tag=f"a{i}", name=f"apt{i}")
                        for j in range(6):
                            nc.tensor.matmul(
                                ap_t[:],
                                g_s[:, j, 128 * i : 128 * (i + 1)],
                                wqt_s[:, j, hsl],
                                start=(j == 0), stop=(j == 5),
                            )
                        if i % 2 == 0:
                            nc.vector.tensor_copy(a_s[:, i, :], ap_t[:])
                        else:
                            nc.scalar.copy(a_s[:, i, :], ap_t[:])
                with tc.tile_pool(name=f"ps_sc{h}", bufs=1, space="PSUM") as ps_sc:
                    st_lo = ps_sc.tile([128, HD], f32, name="st_lo")
                    st_hi = ps_sc.tile([64, HD], f32, name="st_hi")
                    for i in range(6):
                        nc.tensor.matmul(
                            st_lo[:], wkt_s[:, i, HD * h : HD * h + 128],
                            a_s[:, i, :],
                            start=(i == 0), stop=(i == 5),
                            skip_group_check=True,
                        )
                    for i in range(6):
                        nc.tensor.matmul(
                            st_hi[:], wkt_s[:, i, HD * h + 128 : HD * (h + 1)],
                            a_s[:, i, :],
                            start=(i == 0), stop=(i == 5),
                            skip_group_check=True,
                        )
                    nc.vector.tensor_copy(sp_lo[:, h, :], st_lo[:])
                    nc.scalar.copy(sp_hi[:, h, :], st_hi[:])
            for h in range(4):
                nc.sync.dma_start(ssend[h, 0:128, :], sp_lo[:, h, :])
                nc.sync.dma_start(ssend[h, 128:HD, :], sp_hi[:, h, :])
            nc.gpsimd.collective_compute(
                "AllReduce",
                mybir.AluOpType.add,
                replica_groups=RG,
                ins=[ssend.opt()],
                outs=[srecv.opt()],
            )

            # ============ Phase V (all windows, covers score-AllReduce) ======
            vtlo_s = vtpool.tile([128, 4, ROWS], bf16, name="vtlo_s")
            vthi_s = vtpool.tile([64, 4, ROWS], bf16, name="vthi_s")

            def v_window(ps_v, w):
                nsl = slice(512 * w, 512 * (w + 1))
                for h in range(4):
                    vlo = ps_v.tile([128, 512], f32, tag="vlo")
                    vhi = ps_v.tile([64, 512], f32, tag="vhi")
                    for cb in range(6):
                        nc.tensor.matmul(
                            vlo[:], wvlo_s[:, cb, 128 * h : 128 * (h + 1)],
                            xht_s[:, cb, nsl],
                            start=(cb == 0), stop=(cb == 5),
                        )
                    for cb in range(6):
                        nc.tensor.matmul(
                            vhi[:], wvhi_s[:, cb, 64 * h : 64 * (h + 1)],
                            xht_s[:, cb, nsl],
                            start=(cb == 0), stop=(cb == 5),
                        )
                    nc.vector.tensor_copy(vtlo_s[:, h, nsl], vlo[:])
                    nc.scalar.copy(vthi_s[:, h, nsl], vhi[:])

            with tc.tile_pool(name="ps_v1", bufs=2, space="PSUM") as ps_v1:
                for w in range(NW):
                    v_window(ps_v1, w)


            # ---- softmax + transpose per head from reduced scores ----
            sr_lo, sr_hi = sp_lo, sp_hi
            for h in range(4):
                nc.sync.dma_start(sr_lo[:, h, :], srecv[h, 0:128, :])
                nc.sync.dma_start(sr_hi[:, h, :], srecv[h, 128:HD, :])
            smg_lo = spool.tile([128, 4, HD], bf16)
            smg_hi = spool.tile([64, 4, HD], bf16)
            for h in range(4):
                smt = {}
                for src_t, nrow in ((sr_lo, 128), (sr_hi, 64)):
                    ap_in = src_t[0:nrow, h, :]
                    mx = spool.tile([nrow, 1], f32, tag=f"mx{nrow}", name="mx")
                    nc.vector.tensor_reduce(
                        mx[:], ap_in,
                        axis=mybir.AxisListType.X, op=mybir.AluOpType.max,
                    )
                    nmt = spool.tile([nrow, 1], f32, tag=f"nmt{nrow}", name="nmt")
                    nc.vector.tensor_mul(nmt[:], mx[:], tvec_s[:nrow, h : h + 1])
                    nc.vector.tensor_scalar_mul(nmt[:], nmt[:], -1.0)
                    p_t = spool.tile([nrow, HD], f32, tag=f"p{nrow}", name="p_t")
                    ssum = spool.tile([nrow, 1], f32, tag=f"s{nrow}", name="ssum")
                    nc.scalar.activation(
                        p_t[:], ap_in,
                        mybir.ActivationFunctionType.Exp,
                        bias=nmt[:], scale=tvec_s[:nrow, h : h + 1],
                        accum_out=ssum[:],
                    )
                    rcp = spool.tile([nrow, 1], f32, tag=f"r{nrow}", name="rcp")
                    nc.vector.reciprocal(rcp[:], ssum[:])
                    smt_t = spool.tile([nrow, HD], f32, tag=f"smt{nrow}", name="smt_t")
                    nc.vector.tensor_scalar_mul(smt_t[:], p_t[:], rcp[:])
                    smt[nrow] = smt_t
                with tc.tile_pool(name=f"ps_smt{h}", bufs=1, space="PSUM") as ps_smt:
                    tlo = ps_smt.tile([128, HD], f32, name="tlo")
                    thi = ps_smt.tile([64, HD], f32, name="thi")
                    nc.tensor.matmul(
                        tlo[:, 0:128], smt[128][:, 0:128], identf[:],
                        is_transpose=True, start=True, stop=True,
                        skip_group_check=True,
                    )
                    nc.tensor.matmul(
                        tlo[:, 128:HD], smt[64][:, 0:128], identf[:64, 0:64],
                        is_transpose=True, start=True, stop=True,
                        skip_group_check=True,
                    )
                    nc.tensor.matmul(
                        thi[:, 0:128], smt[128][:, 128:HD], identf[:],
                        is_transpose=True, start=True, stop=True,
                        skip_group_check=True,
                    )
                    nc.tensor.matmul(
                        thi[:, 128:HD], smt[64][:, 128:HD], identf[:64, 0:64],
                        is_transpose=True, start=True, stop=True,
                        skip_group_check=True,
                    )
                    nc.vector.tensor_copy(smg_lo[:, h, :], tlo[:])
                    nc.vector.tensor_copy(smg_hi[:, h, :], thi[:])

            # ============ Phase X: x_caT = sm^T-contract vT ============
            xc_cm = tc.tile_pool(name="xcpool", bufs=1)
            xcpool = xc_cm.__enter__()
            xclo_s = xcpool.tile([128, 4, ROWS], bf16, name="xclo_s")
            xchi_s = xcpool.tile([64, 4, ROWS], bf16, name="xchi_s")
            with tc.tile_pool(name="ps_x", bufs=2, space="PSUM") as ps_x:
                for w in range(NW):
                    nsl = slice(512 * w, 512 * (w + 1))
                    for h in range(4):
                        xlo = ps_x.tile([128, 512], f32, tag="xlo")
                        xhi = ps_x.tile([64, 512], f32, tag="xhi")
                        nc.tensor.matmul(
                            xlo[:], smg_lo[:, h, 0:128], vtlo_s[:, h, nsl],
                            start=True, stop=False,
                        )
                        nc.tensor.matmul(
                            xlo[:], smg_hi[:, h, 0:128], vthi_s[:, h, nsl],
                            start=False, stop=True,
                        )
                        nc.tensor.matmul(
                            xhi[:], smg_lo[:, h, 128:HD], vtlo_s[:, h, nsl],
                            start=True, stop=False,
                        )
                        nc.tensor.matmul(
                            xhi[:], smg_hi[:, h, 128:HD], vthi_s[:, h, nsl],
                            start=False, stop=True,
                        )
                        nc.vector.tensor_copy(xclo_s[:, h, nsl], xlo[:])
                        nc.scalar.copy(xchi_s[:, h, nsl], xhi[:])

            for i in range(8):
                e0 = 24 * i
                for h in range(4):
                    if e0 + 24 <= 128:
                        nc.sync.dma_start(
                            a2asend[i, h, :, :],
                            xclo_s[e0 : e0 + 24, h, :],
                        )
                    elif e0 >= 128:
                        nc.sync.dma_start(
                            a2asend[i, h, :, :],
                            xchi_s[e0 - 128 : e0 - 128 + 24, h, :],
                        )
                    else:
                        nlo = 128 - e0
                        nc.sync.dma_start(
                            a2asend[i, h, 0:nlo, :],
                            xclo_s[e0:128, h, :],
                        )
                        nc.sync.dma_start(
                            a2asend[i, h, nlo:24, :],
                            xchi_s[0 : 24 - nlo, h, :],
                        )
            nc.gpsimd.collective_compute(
                "AllToAll",
                mybir.AluOpType.bypass,
                replica_groups=[list(range(8))],
                ins=[a2asend.opt()],
                outs=[a2arecv.opt()],
            )
            # vband[b, 4*e_local + h, n] assembled from batch-b senders 4b+j
            for b in range(2):
                vband_v = vband[b].rearrange("(e h) n -> h e n", h=4)
                for j in range(4):
                    nc.sync.dma_start(
                        vband_v[:, :, ROWS * j : ROWS * (j + 1)],
                        a2arecv[4 * b + j],
                    )

            # ============ Phase P: projection + bias ============
            with (
                tc.tile_pool(name="zpool", bufs=2) as zpool,
                tc.tile_pool(name="opool", bufs=1) as opool,
                tc.tile_pool(name="ps_zt", bufs=2, space="PSUM") as ps_zt,
                tc.tile_pool(name="ps_o", bufs=2, space="PSUM") as ps_o,
            ):
                for bt in range(16):
                    b, t = bt // 8, bt % 8
                    tsl = slice(128 * t, 128 * (t + 1))
                    z_nat = zpool.tile([128, C], bf16, tag="zn")
                    nc.sync.dma_start(
                        z_nat[:], vband[b, 12 * t : 12 * (t + 1), :]
                    )
                    ztp = ps_zt.tile([128, C], bf16, tag="ztp")
                    for j in range(6):
                        nc.tensor.matmul(
                            ztp[:, 128 * j : 128 * (j + 1)],
                            z_nat[:, 128 * j : 128 * (j + 1)],
                            identb[:], is_transpose=True,
                            start=True, stop=True, skip_group_check=True,
                        )
                    zt = zpool.tile([128, 6, 128], bf16, tag="zt")
                    if t % 2 == 0:
                        nc.vector.tensor_copy(
                            zt[:], ztp[:].rearrange("p (o f) -> p o f", f=128)
                        )
                    else:
                        nc.scalar.copy(
                            zt[:], ztp[:].rearrange("p (o f) -> p o f", f=128)
                        )
                    o1 = ps_o.tile([128, 384], f32, tag="o1")
                    o2 = ps_o.tile([128, 384], f32, tag="o2")
                    for j in range(6):
                        nc.tensor.matmul(
                            o1[:], zt[:, j, :], wpt_s[:, j, 0:384],
                            start=(j == 0), stop=(j == 5),
                        )
                    for j in range(6):
                        nc.tensor.matmul(
                            o2[:], zt[:, j, :], wpt_s[:, j, 384:C],
                            start=(j == 0), stop=(j == 5),
                        )
                    out_sb = opool.tile([128, C], f32, tag="ob")
                    nc.vector.tensor_add(out_sb[:, 0:384], o1[:], bias_s[:, 0:384])
                    nc.vector.tensor_add(out_sb[:, 384:C], o2[:], bias_s[:, 384:C])
                    nc.sync.dma_start(out_d[b, tsl, :], out_sb[:])

            xc_cm.__exit__(None, None, None)
            sp_cm.__exit__(None, None, None)
            small_cm.__exit__(None, None, None)
            xht_cm.__exit__(None, None, None)
            vt_cm.__exit__(None, None, None)

    nc.compile()
    return nc


def _get_nc():
    if "nc" not in _cached:
        _cached["nc"] = _build()
    return _cached["nc"]


def _prep_in_maps(x, w_qkv, temperature, w_proj, b_proj):
    import ml_dtypes

    bf = ml_dtypes.bfloat16
    x = np.ascontiguousarray(np.asarray(x, dtype=np.float32))
    w_qkv = np.asarray(w_qkv, dtype=np.float32)
    temperature = np.asarray(temperature, dtype=np.float32)
    w_proj = np.asarray(w_proj, dtype=np.float32)
    b_proj = np.asarray(b_proj, dtype=np.float32)

    wv_full = w_qkv[2 * C : 3 * C]  # [768, 768]
    wvT = np.ascontiguousarray(wv_full.T)  # [c_in, (h,d)]
    wvT_r = wvT.reshape(C, H, HD)
    wvlo = np.ascontiguousarray(
        wvT_r[:, :, 0:128].transpose(0, 1, 2).reshape(C, H * 128)
    ).astype(bf)
    wvhi = np.ascontiguousarray(
        wvT_r[:, :, 128:HD].reshape(C, H * 64)
    ).astype(bf)

    wpt = np.ascontiguousarray(w_proj.T).astype(bf)  # [c_in(=4e+h), c_out]
    wqt = np.ascontiguousarray(w_qkv[0:C].T)
    wkt = np.ascontiguousarray(w_qkv[C : 2 * C].T)

    bias = np.ascontiguousarray(np.broadcast_to(b_proj, (128, C)))
    identb = np.eye(128, dtype=np.float32).astype(bf)
    identf = np.eye(128, dtype=np.float32)

    in_maps = []
    for c in range(NCORE):
        b, r = c // 4, c % 4
        h = c % 4
        xs = x[b, ROWS * r : ROWS * (r + 1), :]
        xh = xs.astype(bf)
        xl2 = ((xs - xh.astype(np.float32)) * 2.0).astype(bf)
        tvec = np.broadcast_to(
            temperature.reshape(1, H).astype(np.float32), (128, H)
        ).copy()
        in_maps.append(
            {
                "xh": np.ascontiguousarray(xh),
                "xl2": np.ascontiguousarray(xl2),
                "wvlo": wvlo,
                "wvhi": wvhi,
                "wqt": wqt,
                "wkt": wkt,
                "wpt": wpt,
                "bias": bias,
                "tvec": tvec,
                "identb": identb,
                "identf": identf,
            }
        )
    return in_maps


def kernel(x, w_qkv, temperature, w_proj, b_proj):
    from concourse.bass_utils import run_bass_kernel_spmd

    nc = _get_nc()
    in_maps = _prep_in_maps(x, w_qkv, temperature, w_proj, b_proj)
    res = run_bass_kernel_spmd(nc, in_maps, core_ids=list(range(NCORE)))
    out = np.empty((B, N, C), np.float32)
    for c in range(NCORE):
        o = res.results[c]["out"]  # [2, 1024, C]
        for b in range(B):
            out[b, 1024 * c : 1024 * (c + 1), :] = o[b]
    return out

